# revision 20
# baseline (speedup 1.0000x reference)
"""Trainium2 Bass kernel for nn_Attention (B=2, N=4096, D=1024, 16 heads).

Sharding: 8 cores = 2 (batch) x 4 (head groups of 4 heads, Megatron TP).
Partial projections summed on host (the TP all-reduce), plus bias.

v4: dual-engine softmax, all-fp16 datapath.
 - per j-tile, the two heads' scores land in separate PSUM tiles; ACT
   computes exact spline exp for the even head while a runtime-registered
   8-stage custom DVE op computes (C0*(s+30)^2 + C1)^32 ~= exp(s) for the
   odd head concurrently. The +30 vertex shift rides a 65th contraction row
   (k=4.0, q=7.5) on the odd head's QK only -- contraction >64 costs 2x on
   the PE, so the even head contracts 64 deep.
 - PV is plain fp16 per key-tile with a ones-column (65 output partitions,
   which are free) yielding the softmax denominator on PSUM row 64; matmul
   throughput here is PSUM-column-write-bound, so fp8 DoubleRow buys
   nothing and fp16 keeps full accuracy.
 - normalization happens on eviction (DVE multiply by gpsimd-broadcast
   reciprocal); PSUM-side copies ride the ACT engine; projection is fp16.
"""

from collections import deque

import numpy as np

import concourse.bacc as bacc
import concourse.mybir as mybir
import concourse.tile as tile
import concourse.dve_ops as dve_ops
from concourse.dve_spec import Spec, Src0, C0, C1, sq, lower
from concourse.dve_uop import DveOpSpec

B = 2
N = 4096
D = 1024
HL = 4          # heads per core
HD = 64         # head dim
DG = HL * HD    # 256 = per-core d' width
DT = 8          # contraction tiles for qkv
NT = 32         # key tiles of 128
NJP = 16        # key tile pairs
NCH = 8         # query chunks of 512

AEXP = 30.0     # score shift from const row (4.0 * 7.5), DVE heads only
C0EXP = 4.5966988670e-04   # (C0*(s+30)^2+C1)^32 ~= exp(s-4)
C1EXP = 0.46874890
ACT_BIAS = -4.0            # exp(s-4): keeps fp8 weights under 240

FP32 = mybir.dt.float32
FP16 = mybir.dt.float16
FP8 = mybir.dt.float8e4
MULT = mybir.AluOpType.mult
SUB = mybir.AluOpType.subtract
EXPF = mybir.ActivationFunctionType.Exp
DR = mybir.MatmulPerfMode.DoubleRow


def _register_exp32():
    """Register the 8-stage approx-exp op into the custom-DVE catalog.

    body: u = sq(Src0)*C0 + C1; out = u^32 (5 squarings). Exactly fills the
    8 ALU stages. Src0 must arrive pre-shifted (s + 30)."""
    name = "EXP32ANT"
    for op in dve_ops.OPS:
        if op.name == name:
            return op
    body = sq(Src0) * C0 + C1
    for _ in range(5):
        body = sq(body)

    def ref(in0, in1, c0, c1, c2):
        u = (in0.astype(np.float32) ** 2) * c0 + c1
        for _ in range(5):
            u = u * u
        return u

    spec = Spec(body=body, reference=ref)
    row = dve_ops._CUSTOM_DVE_ROW_BASE + len(dve_ops.OPS)
    shas = {}
    for ver in ("v3", "v4"):
        uops = lower(spec, ver=ver)
        shas[ver] = DveOpSpec(name=name, opcode=row, uops=uops,
                              rd1_en=False).sha(ver)
    op = dve_ops.DveOp(name=name, spec=spec, subdim=False, uops_sha=shas)
    dve_ops.OPS.append(op)
    dve_ops._SUB_OPCODE_FOR_NAME[name] = row
    dve_ops.CUSTOM_DVE_SPECS[name] = spec
    return op


EXP32 = _register_exp32()


def _build(n=N):
    nc = bacc.Bacc("TRN2", target_bir_lowering=False, debug=False)

    xT = nc.declare_dram_parameter("xT", [D, n], FP16, isOutput=False)
    wqT = nc.declare_dram_parameter("wqT", [D, DG], FP16, isOutput=False)
    wkT = nc.declare_dram_parameter("wkT", [D, DG], FP16, isOutput=False)
    wvT = nc.declare_dram_parameter("wvT", [D, DG], FP16, isOutput=False)
    wpT2 = nc.declare_dram_parameter("wpT2", [128, 2, D], FP16, isOutput=False)
    kcst = nc.declare_dram_parameter("kcst", [1, HL, n], FP16, isOutput=False)
    qcst = nc.declare_dram_parameter("qcst", [1, 2, HL, 512], FP16,
                                     isOutput=False)
    out = nc.declare_dram_parameter("out", [n, D], FP16, isOutput=True)

    xT_r = xT.rearrange("(dt p) n -> dt p n", p=128)
    wq_r = wqT.rearrange("(dt p) m -> dt p m", p=128)
    wk_r = wkT.rearrange("(dt p) m -> dt p m", p=128)
    wv_r = wvT.rearrange("(dt p) m -> dt p m", p=128)

    with tile.TileContext(nc) as tc:
        with (
            tc.tile_pool(name="sb", bufs=1) as sb,
            tc.tile_pool(name="wkp", bufs=1) as wkpool,
            tc.tile_pool(name="ps", bufs=1, space="PSUM") as ps,
        ):
            # ---- persistent SBUF tiles ----
            xt = sb.tile([128, DT, n], FP16, tag="xt")
            wq_t = sb.tile([128, DT, DG], FP16, tag="wq")
            wk_t = sb.tile([128, DT, DG], FP16, tag="wk")
            wv_t = sb.tile([128, DT, DG], FP16, tag="wv")
            wp_t = sb.tile([128, 2, D], FP16, tag="wp")
            kt2 = sb.tile([128, HL, n], FP16, tag="kt2")
            qtc = sb.tile([128, 2, HL, 512], FP16, tag="qtc")
            vaug = sb.tile([128, NT, 2, HL, 80], FP8, tag="vaug")
            otn = sb.tile([128, 2, 2, 512], FP16, tag="otn")
            bt = sb.tile([128, 1], FP32, tag="bias")
            nc.vector.memset(bt[:, :], ACT_BIAS)

            # ---- weight + const loads (wk + first x chunk first) ----
            for dt_i in range(DT):
                nc.sync.dma_start(wk_t[:, dt_i, :], wk_r[dt_i, :, :])
                nc.sync.dma_start(xt[:, dt_i, 0:512], xT_r[dt_i, :, 0:512])
            for dt_i in range(DT):
                nc.sync.dma_start(wv_t[:, dt_i, :], wv_r[dt_i, :, :])
            for dt_i in range(DT):
                nc.sync.dma_start(wq_t[:, dt_i, :], wq_r[dt_i, :, :])
            nc.sync.dma_start(wp_t[:, :, :], wpT2[:, :, :])
            # zero rows 64-127 first (full 128-row QK tiles run 2x faster
            # than partial 65-row tiles); the const DMAs then set row 64
            nc.gpsimd.memset(kt2[64:128, :, :], 0.0)
            nc.gpsimd.memset(qtc[64:128, :, :, :], 0.0)
            nc.sync.dma_start(kt2[64:65, :, :], kcst[:, :, :])
            nc.sync.dma_start(qtc[64:65, :, :, :], qcst[:, :, :, :])
            for jt in range(NT):
                nc.vector.memset(vaug[:, jt, 0, :, 64], 1.0)
                nc.vector.memset(vaug[:, jt, 1, :, 64], 0.0)


            # ---- background machinery: bgq = urgent (Q gen), bg = rest ----
            bgq = deque()
            bg = deque()

            def drain(k=1):
                done = 0
                while (bgq or bg) and done < k:
                    q = bgq if bgq else bg
                    try:
                        next(q[0])
                        done += 1
                    except StopIteration:
                        q.popleft()

            def drain_all(q):
                while q:
                    try:
                        next(q[0])
                    except StopIteration:
                        q.popleft()

            # ---- producers ----
            def qgen(c):
                cb = c % 2
                cs = slice(c * 512, (c + 1) * 512)
                for m in range(2):
                    qp = ps.tile([128, 512], FP32, tag="ste", bufs=3,
                                 name="qp")
                    for dt_i in range(DT):
                        nc.tensor.matmul(
                            qp[:, :], wq_t[:, dt_i, m * 128:(m + 1) * 128],
                            xt[:, dt_i, cs],
                            start=(dt_i == 0), stop=(dt_i == DT - 1))
                    qs = wkpool.tile([128, 512], FP16, tag="ksb", bufs=2,
                                     name="qs")
                    nc.vector.tensor_copy(qs[:, :], qp[:, :])
                    nc.sync.dma_start(qtc[0:64, cb, 2 * m, :], qs[0:64, :])
                    nc.sync.dma_start(qtc[0:64, cb, 2 * m + 1, :],
                                      qs[64:128, :])
                    yield

            def kgen(c8):
                cs = slice(c8 * 512, (c8 + 1) * 512)
                for m in range(2):
                    kp = ps.tile([128, 512], FP32, tag="ste" if m == 0
                                 else "sto", bufs=3, name="kp")
                    for dt_i in range(DT):
                        nc.tensor.matmul(
                            kp[:, :], wk_t[:, dt_i, m * 128:(m + 1) * 128],
                            xt[:, dt_i, cs],
                            start=(dt_i == 0), stop=(dt_i == DT - 1))
                    ks = wkpool.tile([128, 512], FP16, tag="ksb", bufs=2,
                                     name="ks")
                    nc.vector.tensor_copy(ks[:, :], kp[:, :])
                    nc.sync.dma_start(kt2[0:64, 2 * m, cs], ks[0:64, :])
                    nc.sync.dma_start(kt2[0:64, 2 * m + 1, cs],
                                      ks[64:128, :])

            def vgen(c8):
                # 4 key tiles (512 keys) per chunk; 2 per aux psum tile
                for half in range(2):
                    yield
                    vp = ps.tile([128, 512], FP32, tag="ste" if half == 0
                                 else "sto", bufs=3, name="vp")
                    for sub in range(2):
                        jt = c8 * 4 + half * 2 + sub
                        for dt_i in range(DT):
                            nc.tensor.matmul(
                                vp[:, sub * 256:(sub + 1) * 256],
                                xt[:, dt_i, jt * 128:(jt + 1) * 128],
                                wv_t[:, dt_i, :],
                                start=(dt_i == 0), stop=(dt_i == DT - 1))
                    for sub in range(2):
                        jt = c8 * 4 + half * 2 + sub
                        src = vp[:, sub * 256:(sub + 1) * 256]
                        vhi = wkpool.tile([128, 256], FP8, tag="vsb",
                                          bufs=4, name="vhi")
                        vlo = wkpool.tile([128, 256], FP8, tag="vsb",
                                          bufs=4, name="vlo")
                        nc.vector.tensor_copy(vhi[:, :], src)
                        nc.vector.tensor_tensor(vlo[:, :], src, vhi[:, :],
                                                SUB)
                        nc.sync.dma_start(vaug[:, jt, 0, :, 0:64],
                                          vhi[:, :])
                        nc.sync.dma_start(vaug[:, jt, 1, :, 0:64],
                                          vlo[:, :])

            # ---- normalize + projection background generators ----
            def norm(h, o65, c, p):
                # o65: SBUF fp32 copy of the [65, 512] PSUM accumulator
                cb = c % 2
                pt, odd = h // 2, h % 2
                zr = wkpool.tile([1, 512], FP32, tag="zr", bufs=4, name="zr")
                nc.scalar.copy(zr[:, :], o65[64:65, :])
                yield
                rz = wkpool.tile([1, 512], FP32, tag="rz", bufs=4, name="rz")
                nc.vector.reciprocal_approx_fast(rz[:, :], zr[:, :])
                yield
                rzs = wkpool.tile([64, 512], FP32, tag="rzs", bufs=4,
                                  name="rzs")
                nc.gpsimd.partition_broadcast(rzs[:, :], rz[:, :])
                yield
                if odd == 0:
                    nc.vector.tensor_tensor(otn[0:64, cb, pt, :],
                                            o65[0:64, :], rzs[:, :], MULT)
                    yield
                else:
                    osb = wkpool.tile([64, 512], FP16, tag="osb", bufs=4,
                                      name="osb")
                    nc.vector.tensor_tensor(osb[:, :], o65[0:64, :],
                                            rzs[:, :], MULT)
                    yield
                    nc.sync.dma_start(otn[64:128, cb, pt, :], osb[:, :])
                    yield

            def proj(c):
                cb = c % 2
                for isub in range(4):
                    ib = c * 512 + isub * 128
                    for e in range(2):
                        pj = ps.tile([128, 512], FP32, tag="sto", bufs=3,
                                     name="pj")
                        for pt in range(2):
                            nc.tensor.matmul(
                                pj[:, :],
                                otn[:, cb, pt,
                                    isub * 128:(isub + 1) * 128],
                                wp_t[:, pt, e * 512:(e + 1) * 512],
                                start=(pt == 0), stop=(pt == 1))
                        ob = wkpool.tile([128, 512], FP16, tag="ob", bufs=3,
                                         name="ob")
                        nc.vector.tensor_copy(ob[:, :], pj[:, :])
                        nc.sync.dma_start(
                            out[ib:ib + 128, e * 512:(e + 1) * 512],
                            ob[:, :])
                        yield

            # ---- prefix: x load + K + V per chunk, then Q(0) ----
            for c8 in range(NCH):
                cs = slice(c8 * 512, (c8 + 1) * 512)
                if c8 > 0:
                    for dt_i in range(DT):
                        nc.sync.dma_start(xt[:, dt_i, cs], xT_r[dt_i, :, cs])
                kgen(c8)
                if c8 < 2:
                    for _ in vgen(c8):
                        pass
                else:
                    bgq.append(vgen(c8))
            for _ in qgen(0):
                pass

            # ---- attention backbone ----
            for c in range(NCH):
                if c > 0:
                    drain_all(bgq)   # qtc for this chunk fully emitted
                if c < NCH - 1:
                    bgq.append(qgen(c + 1))
                for p in range(2):
                    he, ho = 2 * p, 2 * p + 1
                    cb = c % 2
                    ote = ps.tile([65, 512], FP32, tag="ote", bufs=1,
                                  name="ote")
                    oto = ps.tile([65, 512], FP32, tag="oto", bufs=1,
                                  name="oto")
                    pend = None
                    for j in range(NT):
                        ste = ps.tile([128, 512], FP32, tag="ste", bufs=3,
                                      name="ste")
                        sto = ps.tile([128, 512], FP32, tag="sto", bufs=3,
                                      name="sto")
                        # he: 64-deep contraction (no shift; exact ACT exp)
                        nc.tensor.matmul(
                            ste[:, :],
                            kt2[0:64, he, j * 128:(j + 1) * 128],
                            qtc[0:64, cb, he, :], start=True, stop=True)
                        # ho: 128-deep (row 64 adds +30 for the DVE op;
                        # rows 65-127 are zeros -- full row-tile runs 2x
                        # faster than a partial 65-row tile)
                        nc.tensor.matmul(
                            sto[:, :],
                            kt2[:, ho, j * 128:(j + 1) * 128],
                            qtc[:, cb, ho, :], start=True, stop=True)
                        etj = sb.tile([128, 2, 512], FP8, tag="et",
                                      bufs=3, name="etj")
                        # he-exp on ACT (exact), ho-exp on DVE (approx);
                        # the two run concurrently on separate st halves
                        nc.scalar.activation(etj[:, 0, :], ste[:, :],
                                             EXPF, bias=bt[:, :], scale=1.0)
                        nc.vector._custom_dve(EXP32,
                                              out=etj[:, 1, :],
                                              in0=sto[:, :],
                                              s0=C0EXP, s1=C1EXP)
                        if pend is not None:
                            pj_, pet = pend
                            for h, ot in ((he, ote), (ho, oto)):
                                # one DoubleRow matmul: v_hi and v_lo ride
                                # the two k-slots; the rhs 0-stride
                                # broadcast reads the same weights for both
                                nc.tensor.matmul(
                                    ot[0:65, :],
                                    vaug[:, pj_, :, h, 0:65],
                                    pet[:, h % 2:h % 2 + 1, :]
                                    .broadcast_to([128, 2, 512]),
                                    start=(pj_ == 0), stop=False,
                                    perf_mode=DR)
                        pend = (j, etj)
                        drain(1 + (j % 2))
                    pj_, pet = pend
                    for h, ot in ((he, ote), (ho, oto)):
                        nc.tensor.matmul(
                            ot[0:65, :], vaug[:, pj_, :, h, 0:65],
                            pet[:, h % 2:h % 2 + 1, :]
                            .broadcast_to([128, 2, 512]),
                            start=False, stop=True, perf_mode=DR)
                    # evict accumulators to SBUF inline (frees PSUM for the
                    # next block); the rest of normalize runs in background
                    o65e = wkpool.tile([65, 512], FP32, tag="o65", bufs=4,
                                       name="o65e")
                    o65o = wkpool.tile([65, 512], FP32, tag="o65", bufs=4,
                                       name="o65o")
                    nc.scalar.copy(o65e[:, :], ote[:, :])
                    nc.scalar.copy(o65o[:, :], oto[:, :])
                    if c == NCH - 1:
                        drain_all(bgq)
                        drain_all(bg)
                        for _ in norm(he, o65e, c, p):
                            pass
                        for _ in norm(ho, o65o, c, p):
                            pass
                    else:
                        bg.append(norm(he, o65e, c, p))
                        bg.append(norm(ho, o65o, c, p))
                if c == NCH - 1:
                    for _ in proj(c):
                        pass
                else:
                    bg.append(proj(c))

            drain_all(bgq)
            drain_all(bg)

    nc.compile()
    return nc


_CACHED = {}


def _get_nc(n=N):
    if n not in _CACHED:
        _CACHED[n] = _build(n)
    return _CACHED[n]


def _make_in_maps(x, w_qkv, w_proj):
    f16 = np.float16
    in_maps = []
    kcst = np.full((1, HL, N), 4.0, dtype=f16)
    qcst = np.full((1, 2, HL, 512), 7.5, dtype=f16)
    for c in range(8):
        b, g = divmod(c, 4)
        s = slice(g * DG, (g + 1) * DG)
        wp = w_proj[:, s]  # [D(e), 256]
        in_maps.append({
            "xT": np.ascontiguousarray(x[b].T).astype(f16),
            "wqT": np.ascontiguousarray(
                (w_qkv[0 * D:1 * D][s, :] / 8.0).T).astype(f16),
            "wkT": np.ascontiguousarray(w_qkv[1 * D:2 * D][s, :].T).astype(f16),
            "wvT": np.ascontiguousarray(w_qkv[2 * D:3 * D][s, :].T).astype(f16),
            "wpT2": np.ascontiguousarray(
                wp.T.reshape(2, 128, D).transpose(1, 0, 2)).astype(f16),
            "kcst": kcst,
            "qcst": qcst,
        })
    return in_maps


def _host_fix_rows(full, x, w_qkv, w_proj, b_proj):
    """Exact fp32 recompute for any query rows that hit fp8 overflow
    (softmax weight > 240 -> Inf -> NaN). Data-dependent and rare."""
    bad = np.argwhere(~np.isfinite(full).all(axis=-1))
    if not len(bad):
        return full
    x = np.asarray(x, np.float32)
    w = np.asarray(w_qkv, np.float32)
    wp = np.asarray(w_proj, np.float32)
    for b in np.unique(bad[:, 0]):
        rows = bad[bad[:, 0] == b][:, 1]
        xb = x[b]
        k = xb @ w[D:2 * D].T
        v = xb @ w[2 * D:3 * D].T
        q = xb[rows] @ w[0:D].T
        att = np.zeros((len(rows), D), np.float32)
        for h in range(16):
            hs = slice(h * 64, (h + 1) * 64)
            s = (q[:, hs] * (HD ** -0.5)) @ k[:, hs].T
            s -= s.max(axis=1, keepdims=True)
            e = np.exp(s)
            e /= e.sum(axis=1, keepdims=True)
            att[:, hs] = e @ v[:, hs]
        full[b, rows] = att @ wp.T + np.asarray(b_proj, np.float32)
    return full


def kernel(x, w_qkv, w_proj, b_proj):
    from concourse.bass_utils import run_bass_kernel_spmd

    nc = _get_nc(N)
    in_maps = _make_in_maps(np.asarray(x), np.asarray(w_qkv),
                            np.asarray(w_proj))
    res = run_bass_kernel_spmd(nc, in_maps, core_ids=list(range(8)))
    outs = [r["out"].astype(np.float32) for r in res.results]
    full = np.stack([outs[0] + outs[1] + outs[2] + outs[3],
                     outs[4] + outs[5] + outs[6] + outs[7]])
    full += np.asarray(b_proj, dtype=np.float32)[None, None, :]
    full = _host_fix_rows(full, x, w_qkv, w_proj, b_proj)
    return full.astype(np.float32)


# revision 21
# speedup vs baseline: 1.1504x; 1.1504x over previous
"""Trainium2 Bass kernel for nn_Attention (B=2, N=4096, D=1024, 16 heads).

Sharding: 8 cores = 2 (batch) x 4 (head groups of 4 heads, Megatron TP).
Each core computes qkv for its 4 heads, flash-style attention (S^T layout,
softmax denominator via a ones-column folded into the V stationary), and its
partial output projection. The 4 partial projections per batch are summed on
the host during unshard (the TP all-reduce), plus the bias.

v2: software-pipelined instruction emission. The attention j-loop is the
backbone (ACT exp-bound, ~1.06us/iter); everything else (residual QKV
matmul groups, softmax normalization, output projection) is emitted as
background micro-steps interleaved into subsequent j-loops so the PE/ACT
streams never stall at block boundaries. PSUM accumulators are freed at
j-loop end by a single DVE copy to SBUF; the normalize chain (recip ->
gpsimd partition_broadcast -> multiply) runs off the critical path.
"""

from collections import deque

import numpy as np

import concourse.bacc as bacc
import concourse.mybir as mybir
import concourse.tile as tile

B = 2
N = 4096
D = 1024
HL = 4          # heads per core
HD = 64         # head dim
DG = HL * HD    # 256 = per-core d' width
SCALE = HD ** -0.5

FP32 = mybir.dt.float32
BF16 = mybir.dt.bfloat16
MULT = mybir.AluOpType.mult
EXP = mybir.ActivationFunctionType.Exp


def _build(n=N):
    nc = bacc.Bacc("TRN2", target_bir_lowering=False, debug=False)

    xT = nc.declare_dram_parameter("xT", [D, n], BF16, isOutput=False)
    wqT = nc.declare_dram_parameter("wqT", [D, DG], BF16, isOutput=False)
    wkT = nc.declare_dram_parameter("wkT", [D, DG], BF16, isOutput=False)
    wvT = nc.declare_dram_parameter("wvT", [D, DG], BF16, isOutput=False)
    wpT2 = nc.declare_dram_parameter("wpT2", [128, 2, D], BF16, isOutput=False)
    out = nc.declare_dram_parameter("out", [n, D], FP32, isOutput=True)

    DT = D // 128        # 8 contraction tiles for qkv
    NT = n // 128        # key tiles
    QC = min(1024, n)    # qkv prefix group width
    NQC = n // QC
    NC = n // 512        # attention i-chunks

    xT_r = xT.rearrange("(dt p) n -> dt p n", p=128)

    with tile.TileContext(nc) as tc:
        with (
            tc.tile_pool(name="sb", bufs=1) as sb,
            tc.tile_pool(name="wkp", bufs=1) as wkpool,
            tc.tile_pool(name="ps", bufs=1, space="PSUM") as ps,
        ):
            # ---- persistent SBUF tiles ----
            xt = sb.tile([128, DT, n], BF16, tag="xt")
            wq_t = sb.tile([128, DT, DG], BF16, tag="wq")
            wk_t = sb.tile([128, DT, DG], BF16, tag="wk")
            wv_t = sb.tile([128, DT, DG], BF16, tag="wv")
            wp_t = sb.tile([128, 2, D], BF16, tag="wp")
            qt = sb.tile([128, 2, n], BF16, tag="qt")
            kt = sb.tile([128, 2, n], BF16, tag="kt")
            vaug = sb.tile([128, NT, HL, 65], BF16, tag="vaug")
            otn = sb.tile([128, 2, n], BF16, tag="otn")

            # ---- load weights + x^T ----
            wqT_r = wqT.rearrange("(dt p) m -> dt p m", p=128)
            wkT_r = wkT.rearrange("(dt p) m -> dt p m", p=128)
            wvT_r = wvT.rearrange("(dt p) m -> dt p m", p=128)
            for dt_i in range(DT):
                nc.sync.dma_start(wk_t[:, dt_i, :], wkT_r[dt_i, :, :])
            for dt_i in range(DT):
                nc.sync.dma_start(xt[:, dt_i, 0:QC], xT_r[dt_i, :, 0:QC])
            for dt_i in range(DT):
                nc.sync.dma_start(wv_t[:, dt_i, :], wvT_r[dt_i, :, :])
                nc.sync.dma_start(wq_t[:, dt_i, :], wqT_r[dt_i, :, :])
            nc.sync.dma_start(wp_t[:, :, :], wpT2[:, :, :])
            for nh in range(1, NQC):
                for dt_i in range(DT):
                    nc.sync.dma_start(xt[:, dt_i, nh * QC:(nh + 1) * QC],
                                      xT_r[dt_i, :, nh * QC:(nh + 1) * QC])
            for j in range(NT):
                nc.vector.memset(vaug[:, j, :, 64], 1.0)

            # ---- background micro-step machinery ----
            bg = deque()

            def drain(k):
                done = 0
                while bg and done < k:
                    try:
                        next(bg[0])
                        done += 1
                    except StopIteration:
                        bg.popleft()

            # ---- QKV building blocks ----
            def qk_group_wide(w_t, dst, m, c):
                # [128, QC] group on the st tag (prefix only)
                kp = ps.tile([128, QC], FP32, tag="st", bufs=2, name="qkp")
                for dt_i in range(DT):
                    lhs = w_t[:, dt_i, m * 128:(m + 1) * 128]
                    for h2 in range(QC // 512):
                        nc.tensor.matmul(
                            kp[:, h2 * 512:(h2 + 1) * 512],
                            lhs,
                            xt[:, dt_i, c * QC + h2 * 512:
                               c * QC + (h2 + 1) * 512],
                            start=(dt_i == 0), stop=(dt_i == DT - 1),
                        )
                nc.vector.tensor_copy(dst[:, m, c * QC:(c + 1) * QC],
                                      kp[:, :])

            def qk_group_bg(w_t, dst, m, c5):
                # [128, 512] background group on the aux tag
                kp = ps.tile([128, 512], FP32, tag="aux", bufs=2, name="qkb")
                for dt_i in range(DT):
                    nc.tensor.matmul(
                        kp[:, :],
                        w_t[:, dt_i, m * 128:(m + 1) * 128],
                        xt[:, dt_i, c5 * 512:(c5 + 1) * 512],
                        start=(dt_i == 0), stop=(dt_i == DT - 1),
                    )
                    yield
                nc.vector.tensor_copy(dst[:, m, c5 * 512:(c5 + 1) * 512],
                                      kp[:, :])
                yield

            def v_group(j):
                vp = ps.tile([128, DG], FP32, tag="aux", bufs=2, name="vp")
                for dt_i in range(DT):
                    nc.tensor.matmul(
                        vp[:, :],
                        xt[:, dt_i, j * 128:(j + 1) * 128],
                        wv_t[:, dt_i, :],
                        start=(dt_i == 0), stop=(dt_i == DT - 1),
                    )
                for h in range(HL):
                    nc.vector.tensor_copy(vaug[:, j, h, 0:64],
                                          vp[:, h * 64:(h + 1) * 64])

            # ---- normalize + projection generators ----
            def norm_rest(osb, zrow, hh, c):
                pt, odd = hh // 2, hh % 2
                rz = wkpool.tile([1, 512], FP32, tag="rz", bufs=4, name="rz")
                nc.vector.reciprocal_approx_fast(rz[:, :], zrow[:, :])
                yield
                rzs = wkpool.tile([64, 512], FP32, tag="rzs", bufs=4,
                                  name="rzs")
                nc.gpsimd.partition_broadcast(rzs[:, :], rz[:, :])
                yield
                cs = slice(c * 512, (c + 1) * 512)
                if odd == 0:
                    nc.vector.tensor_tensor(otn[0:64, pt, cs],
                                            osb[0:64, :], rzs[:, :], MULT)
                    yield
                else:
                    ohst = wkpool.tile([64, 512], BF16, tag="ohst", bufs=4,
                                       name="ohst")
                    nc.vector.tensor_tensor(ohst[:, :], osb[0:64, :],
                                            rzs[:, :], MULT)
                    yield
                    nc.sync.dma_start(otn[64:128, pt, cs], ohst[:, :])
                    yield

            def proj_gen(c):
                for isub in range(4):
                    ib = c * 512 + isub * 128
                    for e in range(2):
                        pj = ps.tile([128, 512], FP32, tag="aux", bufs=2,
                                     name="pj")
                        for pt in range(2):
                            nc.tensor.matmul(
                                pj[:, :],
                                otn[:, pt, ib:ib + 128],
                                wp_t[:, pt, e * 512:(e + 1) * 512],
                                start=(pt == 0), stop=(pt == 1))
                            yield
                        ob = wkpool.tile([128, 512], FP32, tag="ob", bufs=3,
                                         name="ob")
                        nc.vector.tensor_copy(ob[:, :], pj[:, :])
                        nc.sync.dma_start(
                            out[ib:ib + 128, e * 512:(e + 1) * 512],
                            ob[:, :])
                        yield

            # ---- QKV prefix: K (all), V (all), Q (first QC cols both segs)
            for m in range(2):
                for c in range(NQC):
                    qk_group_wide(wk_t, kt, m, c)
            for j in range(NT):
                v_group(j)
            qk_group_wide(wq_t, qt, 0, 0)
            qk_group_wide(wq_t, qt, 1, 0)
            # remaining Q as background (512-wide groups)
            for c5 in range(QC // 512, NC):
                bg.append(qk_group_bg(wq_t, qt, 0, c5))
                bg.append(qk_group_bg(wq_t, qt, 1, c5))

            # ---- attention backbone ----
            for c in range(NC):
                for p in range(2):
                    he, ho = 2 * p, 2 * p + 1
                    ot_e = ps.tile([128, 512], FP32, tag="ot", bufs=2,
                                   name="ot_e")
                    ot_o = ps.tile([128, 512], FP32, tag="ot", bufs=2,
                                   name="ot_o")
                    pend = None
                    for j in range(NT):
                        st = ps.tile([128, 1024], FP32, tag="st", bufs=2,
                                     name="st")
                        nc.tensor.matmul(
                            st[:, 0:512],
                            kt[0:64, p, j * 128:(j + 1) * 128],
                            qt[0:64, p, c * 512:(c + 1) * 512],
                            start=True, stop=True)
                        nc.tensor.matmul(
                            st[:, 512:1024],
                            kt[64:128, p, j * 128:(j + 1) * 128],
                            qt[64:128, p, c * 512:(c + 1) * 512],
                            start=True, stop=True)
                        et = sb.tile([128, 1024], BF16, tag="et", bufs=3,
                                     name="et", padded_shape=[128, 2048])
                        nc.scalar.activation(et[:, :], st[:, :], EXP,
                                             scale=SCALE)
                        if pend is not None:
                            pj_, pet = pend
                            nc.tensor.matmul(
                                ot_e[0:65, :], vaug[:, pj_, he, 0:65],
                                pet[:, 0:512],
                                start=(pj_ == 0), stop=False)
                            nc.tensor.matmul(
                                ot_o[0:65, :], vaug[:, pj_, ho, 0:65],
                                pet[:, 512:1024],
                                start=(pj_ == 0), stop=False)
                        pend = (j, et)
                        drain(1)
                    pj_, pet = pend
                    nc.tensor.matmul(
                        ot_e[0:65, :], vaug[:, pj_, he, 0:65],
                        pet[:, 0:512], start=False, stop=True)
                    nc.tensor.matmul(
                        ot_o[0:65, :], vaug[:, pj_, ho, 0:65],
                        pet[:, 512:1024], start=False, stop=True)
                    # free the PSUM accumulators with one copy each;
                    # the rest of the normalize chain runs in background
                    for hh, ot_h in ((he, ot_e), (ho, ot_o)):
                        osb = wkpool.tile([64, 512], BF16, tag="osb",
                                          bufs=8, name="osb")
                        nc.vector.tensor_copy(osb[:, :], ot_h[0:64, :])
                        zrow = wkpool.tile([1, 512], FP32, tag="zrow",
                                           bufs=8, name="zrow")
                        nc.vector.tensor_copy(zrow[:, :], ot_h[64:65, :])
                        ng = norm_rest(osb, zrow, hh, c)
                        if c == NC - 1 and p == 1:
                            for _ in ng:
                                pass
                        else:
                            bg.append(ng)
                pg = proj_gen(c)
                if c == NC - 1:
                    while bg:
                        drain(64)
                    for _ in pg:
                        pass
                else:
                    bg.append(pg)

            while bg:
                drain(64)

    nc.compile()
    return nc


_CACHED = {}


def _get_nc(n=N):
    if n not in _CACHED:
        _CACHED[n] = _build(n)
    return _CACHED[n]


def _make_in_maps(x, w_qkv, w_proj):
    import ml_dtypes
    bf16 = ml_dtypes.bfloat16
    in_maps = []
    for c in range(8):
        b, g = divmod(c, 4)
        s = slice(g * DG, (g + 1) * DG)
        wp = w_proj[:, s]  # [D(e), 256]
        in_maps.append({
            "xT": np.ascontiguousarray(x[b].T).astype(bf16),
            "wqT": np.ascontiguousarray(w_qkv[0 * D:1 * D][s, :].T).astype(bf16),
            "wkT": np.ascontiguousarray(w_qkv[1 * D:2 * D][s, :].T).astype(bf16),
            "wvT": np.ascontiguousarray(w_qkv[2 * D:3 * D][s, :].T).astype(bf16),
            "wpT2": np.ascontiguousarray(
                wp.T.reshape(2, 128, D).transpose(1, 0, 2)).astype(bf16),
        })
    return in_maps


def kernel(x, w_qkv, w_proj, b_proj):
    from concourse.bass_utils import run_bass_kernel_spmd

    nc = _get_nc(N)
    in_maps = _make_in_maps(np.asarray(x), np.asarray(w_qkv),
                            np.asarray(w_proj))
    res = run_bass_kernel_spmd(nc, in_maps, core_ids=list(range(8)))
    outs = [r["out"].astype(np.float32) for r in res.results]
    full = np.stack([outs[0] + outs[1] + outs[2] + outs[3],
                     outs[4] + outs[5] + outs[6] + outs[7]])
    full += np.asarray(b_proj, dtype=np.float32)[None, None, :]
    return full.astype(np.float32)

